# revision 23
# baseline (speedup 1.0000x reference)
"""Trainium2 Bass kernel: binarized-MLP forward (784-256-128-32-10, ste_sign).

Strategy
--------
Pure data parallel over 8 NeuronCores: batch 65536 -> 8 shards of 8192 rows;
tiny sign-binarized weights replicated (binarized + packed on the host). Each
core runs the full 4-layer net on its shard feature-major ([features, batch]
tiles, batch as the moving dim) so layer N's output feeds layer N+1 with no
transposes. Outputs leave batch-major via a flipped final matmul.

Layer 1 (the only real-valued matmul; everything downstream only sees
sign(h1)) uses a multi-component split of x whose per-pass scales are folded
into the replicated weight k-rows, so all passes accumulate into ONE PSUM
group with no combine step:

  N8PASS=1: x ~= e4m3(x) + fp16(r1)            (3 B/elem, err ~2^-16|x|)
  N8PASS=2: x ~= e4m3(x) + e4m3(r1*16)/16 + fp16(r2)   (4 B/elem, ~2^-20|x|)

fp8 passes run as DoubleRow matmuls (2 k-tiles per instruction, 0.5
cycles/col -> 4x bf16 throughput); 784 = 98*8 for the fp8 part (4 DoubleRow
pairs per pass, no tail) and 112*7 for the fp16 part (7 full matmuls, no
tail). Layers 2-3 have +-1 inputs and +-1 weights, exact in fp8: activations
are written with feature-halves as DoubleRow slots ([128,2,512] / [64,2,512])
so L2/L3 are single DoubleRow units. L4 is flipped (lhsT = a3 slices, rhs =
w4^T moving, 10-wide): 10 cycles per matmul and batch-major [128,10] PSUM
output, staged into one [128, 640] SBUF tile and DMA'd out in 4 bulk
transfers (partition-major, 2560 B contiguous per partition).

HW Sign(0) = 0, so the integer-valued pre-activations (layers 2,3) use
Sign(h + 0.5), which reproduces the reference's sign(0)=+1 exactly.

This walrus build rejects instructions carrying more than one semaphore wait
("Too many sync wait commands"), so after Tile scheduling, excess waits are
split onto preceding same-engine NoOps (fix_sync_waits).
"""
import sys
sys.path.insert(0, '/opt/trn_rl_repo')
import numpy as np
import ml_dtypes
import concourse.bass as bass
import concourse.mybir as mybir
from concourse import tile
from concourse.bass_utils import run_bass_kernel_spmd

BF16 = ml_dtypes.bfloat16
E4M3 = ml_dtypes.float8_e4m3
F32 = mybir.dt.float32
FP16 = mybir.dt.float16
FP8 = mybir.dt.float8e4
BF = mybir.dt.bfloat16
AF = mybir.ActivationFunctionType
DR = mybir.MatmulPerfMode.DoubleRow

N_CORES = 8
B_LOC = 8192          # batch rows per core
NB = 512              # batch columns per compute chunk (one fp32 PSUM bank)
NCHUNK = B_LOC // NB
N8PASS = 1            # fp8 components of x (1 => +fp16 residual = 3 B/elem)
NT8 = 8 * N8PASS      # fp8 k-tiles of 98 rows
F1, F2, F3, F4 = 256, 128, 32, 10
MAX_WAITS = 1


def fix_sync_waits(nc):
    for fn in nc.m.functions:
        for bb in fn.blocks:
            out = []
            changed = False
            for ins in bb.instructions:
                si = ins.sync_info
                waits = list(si.on_wait) if si is not None else []
                if len(waits) > MAX_WAITS:
                    head, keep = waits[:-MAX_WAITS], waits[-MAX_WAITS:]
                    k = 0
                    while head:
                        chunk, head = head[:MAX_WAITS], head[MAX_WAITS:]
                        nop = mybir.InstNoOp(
                            name=f"{ins.name}-wsplit{k}", engine=ins.engine)
                        nop.sync_info = mybir.SyncInfo(on_wait=chunk, on_update=[])
                        out.append(nop)
                        k += 1
                    ins.sync_info = mybir.SyncInfo(
                        on_wait=keep, on_update=list(si.on_update))
                    changed = True
                out.append(ins)
            if changed:
                bb.instructions = out


def build_nc():
    nc = bass.Bass()
    x8_d = nc.declare_dram_parameter("x8", [98, NT8, B_LOC], FP8, isOutput=False)
    x16_d = nc.declare_dram_parameter("x16", [112, 7, B_LOC], FP16, isOutput=False)
    w8_d = nc.declare_dram_parameter("w8", [98, NT8, F1], FP8, isOutput=False)
    w16_d = nc.declare_dram_parameter("w16", [112, 7, F1], FP16, isOutput=False)
    w2_d = nc.declare_dram_parameter("w2dr", [128, 2, F2], FP8, isOutput=False)
    w3_d = nc.declare_dram_parameter("w3x", [F2, F3], FP8, isOutput=False)
    b3_d = nc.declare_dram_parameter("b3", [F3, 1], F32, isOutput=False)
    w4_d = nc.declare_dram_parameter("w4T", [F3, F4], BF, isOutput=False)
    out_d = nc.declare_dram_parameter("out", [128, NCHUNK * 40], F32,
                                      isOutput=True)

    with tile.TileContext(nc) as tc:
        with tc.tile_pool(name="wpool", bufs=1) as wpool, \
             tc.tile_pool(name="x8pool", bufs=4) as x8pool, \
             tc.tile_pool(name="x16pool", bufs=4) as x16pool, \
             tc.tile_pool(name="apool", bufs=3) as apool, \
             tc.tile_pool(name="ps1", bufs=4, space="PSUM") as ps1, \
             tc.tile_pool(name="ps2", bufs=1, space="PSUM") as ps2, \
             tc.tile_pool(name="ps3", bufs=1, space="PSUM") as ps3, \
             tc.tile_pool(name="ps4", bufs=2, space="PSUM") as ps4:
            # head: land the first DR matmul's operands (w8 pair 0, x8 pair 0)
            # before anything else, split across both HWDGE queues; the fp16
            # pass's operands follow, then the small weights.
            w8t = wpool.tile([98, NT8, F1], FP8, name="w8t")
            nc.sync.dma_start(w8t[:, 0:2, :], w8_d[:, 0:2, :])
            x8t0 = x8pool.tile([98, NT8, NB], FP8, name="x8_0", tag="x8")
            nc.scalar.dma_start(x8t0[:, 0:2, :], x8_d[:, 0:2, :NB])
            nc.sync.dma_start(w8t[:, 2:NT8, :], w8_d[:, 2:NT8, :])
            nc.scalar.dma_start(x8t0[:, 2:NT8, :], x8_d[:, 2:NT8, :NB])
            w16t = wpool.tile([112, 7, F1], FP16, name="w16t")
            x16t0 = x16pool.tile([112, 7, NB], FP16, name="x16_0", tag="x16")
            nc.sync.dma_start(w16t[:, 0:2, :], w16_d[:, 0:2, :])
            nc.scalar.dma_start(x16t0[:, 0:2, :], x16_d[:, 0:2, :NB])
            nc.sync.dma_start(w16t[:, 2:7, :], w16_d[:, 2:7, :])
            nc.scalar.dma_start(x16t0[:, 2:7, :], x16_d[:, 2:7, :NB])
            w2t = wpool.tile([128, 2, F2], FP8, name="w2t")
            nc.scalar.dma_start(w2t[:], w2_d[:, :, :])
            w3t = wpool.tile([F2, F3], FP8, name="w3t")
            nc.scalar.dma_start(w3t[:], w3_d[:, :])
            b3t = wpool.tile([F3, 1], F32, name="b3t")
            nc.scalar.dma_start(b3t[:], b3_d[:, :])
            w4t = wpool.tile([F3, F4], BF, name="w4t")
            nc.scalar.dma_start(w4t[:], w4_d[:, :])
            stage = wpool.tile([128, NCHUNK * 40], F32, name="stage")
            zb = wpool.tile([128, 1], F32, name="zb")
            nc.vector.memset(zb[:], 0.0)
            hb = wpool.tile([128, 1], F32, name="hb")
            nc.vector.memset(hb[:], 0.5)

            for c in range(NCHUNK):
                b0 = c * NB
                if c == 0:
                    x8t, x16t = x8t0, x16t0
                else:
                    x8t = x8pool.tile([98, NT8, NB], FP8, name=f"x8_{c}",
                                      tag="x8")
                    nc.sync.dma_start(x8t[:], x8_d[:, :, b0:b0 + NB])
                    x16t = x16pool.tile([112, 7, NB], FP16, name=f"x16_{c}",
                                        tag="x16")
                    nc.scalar.dma_start(x16t[:], x16_d[:, :, b0:b0 + NB])

                a1t = apool.tile([128, 2, NB], FP8, name=f"a1_{c}", tag="a1")
                for f in range(2):
                    p1 = ps1.tile([128, NB], F32, name=f"p1_{c}_{f}", tag="p1")
                    fs = slice(f * 128, (f + 1) * 128)
                    for j in range(2):
                        js = slice(j * 256, (j + 1) * 256)
                        for u in range(NT8 // 2):
                            nc.tensor.matmul(p1[:, js],
                                             w8t[:, 2 * u:2 * u + 2, fs],
                                             x8t[:, 2 * u:2 * u + 2, js],
                                             start=(u == 0), stop=False,
                                             perf_mode=DR)
                        for i in range(7):
                            nc.tensor.matmul(p1[:, js], w16t[:, i, fs],
                                             x16t[:, i, js],
                                             start=False, stop=(i == 6))
                    nc.scalar.activation(a1t[:, f, :], p1[:], AF.Sign,
                                         bias=zb[:], scale=1.0)

                p2 = ps2.tile([F2, NB], F32, name=f"p2_{c}", tag="p2")
                for j in range(2):
                    js = slice(j * 256, (j + 1) * 256)
                    nc.tensor.matmul(p2[:, js], w2t[:], a1t[:, :, js],
                                     start=True, stop=True, perf_mode=DR)
                # a2 as {0,1} on DVE (is_ge); the 2g-1 affine is folded into
                # w3x = 2*w3s and the a3 bias b3 = 0.5 - rowsum(w3s).
                a2t = apool.tile([F2, NB], FP8, name=f"a2_{c}", tag="a2")
                nc.vector.tensor_scalar(a2t[:], p2[:], 0.0, None,
                                        mybir.AluOpType.is_ge)

                p3 = ps3.tile([F3, NB], F32, name=f"p3_{c}", tag="p3")
                nc.tensor.matmul(p3[:], w3t[:], a2t[:], start=True, stop=True)
                a3t = apool.tile([F3, NB], BF, name=f"a3_{c}", tag="a3")
                nc.scalar.activation(a3t[:], p3[:], AF.Sign, bias=b3t[:],
                                     scale=1.0)

                for sub in range(4):
                    p4 = ps4.tile([128, F4], F32, name=f"p4_{c}_{sub}",
                                  tag="p4")
                    nc.tensor.matmul(p4[:],
                                     a3t[:, sub * 128:(sub + 1) * 128],
                                     w4t[:], start=True, stop=True)
                    nc.vector.tensor_copy(
                        stage[:, c * 40 + sub * 10:c * 40 + (sub + 1) * 10],
                        p4[:])
                if c % 4 == 3:
                    cs = slice((c - 3) * 40, (c + 1) * 40)
                    nc.gpsimd.dma_start(out_d[:, cs], stage[:, cs])
    fix_sync_waits(nc)
    return nc


def _sg(w):
    return np.where(np.asarray(w) >= 0, np.float32(1.0), np.float32(-1.0))


_NC_CACHE = {}


def kernel(x, w1, w2, w3, w4):
    if "nc" not in _NC_CACHE:
        _NC_CACHE["nc"] = build_nc()
    nc = _NC_CACHE["nc"]

    x = np.ascontiguousarray(np.asarray(x).reshape(-1, 784), dtype=np.float32)
    B = x.shape[0]
    w1sT = _sg(w1).T                        # [784, 256]

    # fp8 components of x (pass scales fold into weight rows) + fp16 residual
    xT = np.ascontiguousarray(x.T)          # [784, B]
    comps8 = []
    w8rows = []
    rem = xT
    scale = np.float32(1.0)
    for p in range(N8PASS):
        q = (rem * scale).astype(E4M3)
        comps8.append(q)
        w8rows.append(w1sT / scale)
        rem = rem - q.astype(np.float32) / scale
        scale = np.float32(scale * 16.0)
    x8 = np.stack(comps8, axis=0)           # [N8PASS, 784, B] e4m3
    x8 = np.ascontiguousarray(
        x8.reshape(N8PASS, 8, 98, B).transpose(2, 0, 1, 3).reshape(98, NT8, B))
    w8 = np.stack(w8rows, axis=0)           # [N8PASS, 784, 256]
    w8 = np.ascontiguousarray(
        w8.reshape(N8PASS, 8, 98, F1).transpose(2, 0, 1, 3)
        .reshape(98, NT8, F1)).astype(E4M3)
    x16 = np.ascontiguousarray(
        rem.astype(np.float16).reshape(7, 112, B).transpose(1, 0, 2))
    w16 = np.ascontiguousarray(
        w1sT.reshape(7, 112, F1).transpose(1, 0, 2)).astype(np.float16)

    w2sT = _sg(w2).T                        # [256, 128]
    w2dr = np.ascontiguousarray(
        w2sT.reshape(2, 128, F2).transpose(1, 0, 2)).astype(E4M3)
    w3s = _sg(w3)                           # [32, 128]
    w3x = np.ascontiguousarray(2.0 * w3s.T).astype(E4M3)   # [128, 32]
    b3 = np.ascontiguousarray(
        (0.5 - w3s.sum(axis=1)).reshape(F3, 1)).astype(np.float32)
    w4T = np.ascontiguousarray(_sg(w4).T).astype(BF16)   # [32, 10]

    wm = {"w8": w8, "w16": w16, "w2dr": w2dr, "w3x": w3x, "b3": b3,
          "w4T": w4T}
    maps = []
    for core in range(N_CORES):
        m = dict(wm)
        bs = slice(core * B_LOC, (core + 1) * B_LOC)
        m["x8"] = np.ascontiguousarray(x8[:, :, bs])
        m["x16"] = np.ascontiguousarray(x16[:, :, bs])
        maps.append(m)

    res = None
    last_exc = None
    for attempt in range(3):
        try:
            res = run_bass_kernel_spmd(nc, maps, list(range(N_CORES)))
            break
        except Exception as e:  # transient NRT/device errors: retry
            last_exc = e
            import time
            time.sleep(5 * (attempt + 1))
    if res is None:
        raise last_exc
    # stage layout: [p, c*40 + sub*10 + f]  <->  batch b = c*512 + sub*128 + p
    outs = []
    for r in res.results:
        o = r["out"].reshape(128, NCHUNK, 4, F4)
        outs.append(np.ascontiguousarray(
            o.transpose(1, 2, 0, 3).reshape(B_LOC, F4)))
    return np.ascontiguousarray(np.concatenate(outs, axis=0)).astype(np.float32)
